# revision 50
# baseline (speedup 1.0000x reference)
"""Multi-head attention (B=2, S=2048, D=1024, H=16) on 8 Trainium2 NeuronCores.

Sharding: core c handles batch b = c//4 and head group g = c%4 (4 heads, 256
of the 1024 model dims). All matmul operands are bf16 (host pre-casts X and
weights); PSUM accumulation stays fp32.

Per core:
  kT/qT = (X @ W_{K,Q}[:, g])^T  [128, 2048] bf16 (score scale folded into
          W_Q/b_Q on host). q stored per-head zero-padded to K=128 so the
          scores matmul drives the full PE array with uniform full-array
          LDWEIGHTS pipelining.
  v     =  X @ W_V[:, g] stored [k, i, head, 128] bf16 with ALL-ONES cols
          0:64 and value cols 64:128, so each av matmul yields the softmax
          denominator replicated on psum parts 0:64 (base-0 for the custom
          approx reciprocal) AND attn@v on parts 64:128 (TT ops may read
          shifted PSUM operands).
  exp on ScalarE (no max-subtraction; scores are O(1) by construction).
  drain = single-pass approx reciprocal + per-head multiply writing bf16
          straight into the A2A send tile.
Combine: Q columns host-permuted so attention chunk j covers half of every
core's output rows; per-(chunk, head-pair) 8-core AllToAlls (4 total) start
as soon as each pair drains, overlapping the remaining attention;
full-width bf16 output projection (wrong-batch A2A slots hit zero rows of
the per-core stacked W_O) writes the final [512, 1024] slice. Host
unpermutes.
DMA issue is spread across engines: X/a2a staging on SP, weights/biases on
the ACT hwdge queue, wo/a2l/output on the Pool SWDGE queue.
"""

import sys

if "/opt/trn_rl_repo" not in sys.path:
    sys.path.insert(0, "/opt/trn_rl_repo")

import ml_dtypes
import numpy as np

import concourse.bass as bass
import concourse.mybir as mybir
import concourse.tile as tile
from concourse import bacc
from concourse.bass_utils import run_bass_kernel_spmd

B, S, D = 2, 2048, 1024
H, DK = 16, 64
N_CORES = 8
HPC = 4  # heads per core
EC = HPC * DK  # 256 local model dims per core
F32 = mybir.dt.float32
BF16 = mybir.dt.bfloat16

NJ = 2  # q-chunks of 1024
JW = S // NJ
NI = S // 128  # k-tiles
NP = HPC // 2  # head pairs

# q-column permutation: perm-block r (256 wide) of chunk j = global rows
# [r*512 + j*256 : r*512 + (j+1)*256], so A2A slot r always carries the rows
# core r outputs, half per j-chunk.
_PERM = np.concatenate(
    [np.arange(r * 512 + j * 256, r * 512 + (j + 1) * 256) for j in range(2) for r in range(4)]
)


def _wlayout(w):
    """[1024, EC] -> [128, 8, EC] bf16 matching the SBUF lhsT tile layout."""
    return np.ascontiguousarray(w.reshape(8, 128, EC).transpose(1, 0, 2)).astype(
        ml_dtypes.bfloat16
    )


def _wo_global(W_O):
    """[128, 8, D]: global W_O row-chunks (same for every core)."""
    out = W_O.reshape(8, 128, D).transpose(1, 0, 2)
    return np.ascontiguousarray(out).astype(ml_dtypes.bfloat16)


def _build_nc():
    nc = bacc.Bacc(None, num_devices=N_CORES, num_swdge_queues=4)

    xqt = nc.dram_tensor("xqt", [D, S], BF16, kind="ExternalInput")
    xkt = nc.dram_tensor("xkt", [D, S], BF16, kind="ExternalInput")
    xvt = nc.dram_tensor("xvt", [D, S], BF16, kind="ExternalInput")
    wq = nc.dram_tensor("wq", [128, 8, EC], BF16, kind="ExternalInput")
    wk = nc.dram_tensor("wk", [128, 8, EC], BF16, kind="ExternalInput")
    wv = nc.dram_tensor("wv", [128, 8, EC], BF16, kind="ExternalInput")
    wo = nc.dram_tensor("wo", [128, 8, D], BF16, kind="ExternalInput")
    msel = nc.dram_tensor("msel", [128, 2], F32, kind="ExternalInput")
    bq = nc.dram_tensor("bq", [EC], F32, kind="ExternalInput")
    bk = nc.dram_tensor("bk", [EC], F32, kind="ExternalInput")
    bv = nc.dram_tensor("bv", [EC], F32, kind="ExternalInput")
    bo = nc.dram_tensor("bo", [D], F32, kind="ExternalInput")

    # per-(chunk, head-pair) A2A buffers: slot r carries the pair's 2 heads
    a2a_in = [
        [nc.dram_tensor(f"a2a_in{j}_{p}", [N_CORES, 128, 256], BF16) for p in range(NP)]
        for j in range(NJ)
    ]
    a2a_out = [
        [nc.dram_tensor(f"a2a_out{j}_{p}", [N_CORES, 128, 256], BF16) for p in range(NP)]
        for j in range(NJ)
    ]
    out = nc.dram_tensor("out", [NJ, 256, D], F32, kind="ExternalOutput")

    with tile.TileContext(nc) as tc:
        with (
            tc.tile_pool(name="res", bufs=1) as res,
            tc.tile_pool(name="xt", bufs=10) as xt_pool,
            tc.tile_pool(name="exp", bufs=6) as exp_pool,
            tc.tile_pool(name="osb", bufs=3) as osb_pool,
            tc.tile_pool(name="rec", bufs=4) as rec_pool,
            tc.tile_pool(name="snd", bufs=2) as snd_pool,
            tc.tile_pool(name="a2l", bufs=16) as a2l_pool,
            tc.tile_pool(name="ps", bufs=1, space="PSUM") as ps,
        ):
            # --- weights / biases on the ACT hwdge queue, consumption order ---
            wq_sb = res.tile([128, 8, EC], BF16, tag="wq")
            wk_sb = res.tile([128, 8, EC], BF16, tag="wk")
            wv_sb = res.tile([128, 8, EC], BF16, tag="wv")
            wo_sb = res.tile([128, 8, D], BF16, tag="wo")
            msel_sb = res.tile([128, 2], F32, tag="msel")
            bq_sb = res.tile([128, 2], F32, tag="bq")
            bk_sb = res.tile([128, 2], F32, tag="bk")
            bv_rep = res.tile([128, EC], F32, tag="bv")
            bo_rep = res.tile([128, D], F32, tag="bo")
            nc.scalar.dma_start(out=wk_sb, in_=wk[:])
            nc.scalar.dma_start(out=bk_sb, in_=bk[:].rearrange("(c p) -> p c", p=128))
            nc.scalar.dma_start(out=wq_sb, in_=wq[:])
            nc.scalar.dma_start(out=bq_sb, in_=bq[:].rearrange("(c p) -> p c", p=128))
            nc.scalar.dma_start(out=wv_sb, in_=wv[:])
            nc.scalar.dma_start(out=msel_sb, in_=msel[:])
            nc.scalar.dma_start(
                out=bv_rep,
                in_=bass.AP(tensor=bv[:].tensor, offset=0, ap=[[0, 128], [1, EC]]),
            )

            # --- residents ---
            kt = [res.tile([128, S], BF16, tag=f"kt{c}", name=f"kt{c}") for c in range(2)]
            # per-head q, zero-padded in the complementary 64 partitions so the
            # scores matmul contracts K=128 (uniform full-array matmuls)
            qtz = [
                res.tile([128, S], BF16, tag=f"qtz{h}", name=f"qtz{h}")
                for h in range(HPC)
            ]
            for h in range(HPC):
                z = slice(64, 128) if h % 2 == 0 else slice(0, 64)
                nc.gpsimd.memset(qtz[h][z, :], 0.0)
            # v: ones cols 0:64 (denominator), value cols 64:128
            v_sb = res.tile([128, NI, HPC, 2 * DK], BF16, tag="v")
            nc.gpsimd.memset(v_sb[:, :, :, 0:DK], 1.0)

            # --- k/q projections (X streamed once, full-width DMAs) ---
            # out[e, s] accumulated over d; lhsT = W d-chunk, rhs = X^T.
            for xsrc, w_sb, b_sb, dst in (
                (xkt, wk_sb, bk_sb, kt),
                (xqt, wq_sb, bq_sb, None),
            ):
                pk = [
                    ps.tile([128, 1024], F32, tag="q4", bufs=4, name=f"pk{_c}")
                    for _c in range(4)
                ]
                for d in range(8):
                    xtile = xt_pool.tile([128, S], BF16, tag="xt")
                    nc.sync.dma_start(out=xtile, in_=xsrc[d * 128 : (d + 1) * 128, :])
                    for half in range(2):
                        for c in range(2):
                            for n in range(2):
                                nc.tensor.matmul(
                                    pk[2 * half + c][:, n * 512 : (n + 1) * 512],
                                    w_sb[:, d, c * 128 : (c + 1) * 128],
                                    xtile[
                                        :,
                                        half * 1024 + n * 512 : half * 1024
                                        + (n + 1) * 512,
                                    ],
                                    start=(d == 0),
                                    stop=(d == 7),
                                )
                for half in range(2):
                    hs2 = slice(half * 1024, (half + 1) * 1024)
                    for c in range(2):
                        if dst is not None:
                            nc.vector.tensor_scalar_add(
                                dst[c][:, hs2], pk[2 * half + c], b_sb[:, c : c + 1]
                            )
                        else:
                            nc.vector.tensor_scalar_add(
                                qtz[2 * c][0:64, hs2],
                                pk[2 * half + c][0:64, :],
                                b_sb[0:64, c : c + 1],
                            )
                            nc.vector.tensor_scalar_add(
                                qtz[2 * c + 1][64:128, hs2],
                                pk[2 * half + c][64:128, :],
                                b_sb[64:128, c : c + 1],
                            )

            # --- v projection: natural [s, e]; two passes of 8 s-blocks, one
            # [128, 256] accumulator region per PSUM bank (start=True clears
            # the whole bank's has_written bits, so regions must not share) ---
            for vpass in range(2):
                pvm = [
                    ps.tile([128, 1024], F32, tag="q4", bufs=4, name=f"pv{_m}")
                    for _m in range(4)
                ]
                for d in range(8):
                    xtile = xt_pool.tile([128, S], BF16, tag="xt")
                    nc.sync.dma_start(
                        out=xtile[:, 0:1024],
                        in_=xvt[d * 128 : (d + 1) * 128, vpass * 1024 : (vpass + 1) * 1024],
                    )
                    for m in range(8):
                        nc.tensor.matmul(
                            pvm[m // 2][:, (m % 2) * 512 : (m % 2) * 512 + 256],
                            xtile[:, m * 128 : (m + 1) * 128],
                            wv_sb[:, d, :],
                            start=(d == 0),
                            stop=(d == 7),
                        )
                for m in range(8):
                    nc.vector.tensor_add(
                        v_sb[:, vpass * 8 + m, :, DK : 2 * DK],
                        pvm[m // 2][
                            :, (m % 2) * 512 : (m % 2) * 512 + 256
                        ].rearrange("p (h d) -> p h d", h=HPC),
                        bv_rep.rearrange("p (h d) -> p h d", h=HPC),
                    )

            # wo loads on the Pool queue during attention (doesn't compete
            # with projection-phase x DMAs or the ACT exp stream)
            for ch in range(8):
                nc.gpsimd.dma_start(out=wo_sb[:, ch, :], in_=wo[:, ch, :])

            # --- attention per q-chunk j, A2A per head pair ---
            a2t = {}
            mrg = {}

            def emit_merge(j, p):
                tmp = a2l_pool.tile(
                    [128, 4, 256], BF16, tag="a2l", bufs=8, name=f"tmp{j}_{p}"
                )
                mg = a2l_pool.tile(
                    [128, 4, 256], BF16, tag="mrg", bufs=4, name=f"mrg{j}_{p}"
                )
                mrg[(j, p)] = mg
                a2lo, a2hi = a2t[(j, p)]
                nc.vector.tensor_scalar_mul(tmp, a2hi, msel_sb[:, 1:2])
                nc.vector.affine_then_add(
                    mg, a2lo, tmp, scale=msel_sb[:, 0:1], bias=0.0
                )

            for j in range(NJ):
                # previous chunk's batch-select merges go here: in DVE FIFO
                # order they land right AFTER that chunk's drains (no psum
                # release blocked) and execute in the DVE's idle window while
                # this chunk's attention runs
                if j == 1:
                    for p in range(NP):
                        emit_merge(0, p)
                send = snd_pool.tile([64, HPC, JW], BF16, tag="send", name=f"send{j}")
                for p in range(NP):
                    hA, hB = 2 * p, 2 * p + 1
                    avA = ps.tile([128, 1024], F32, tag="q4", bufs=4)
                    avB = ps.tile([128, 1024], F32, tag="q4", bufs=4)
                    for i in range(NI):
                        isl = slice(i * 128, (i + 1) * 128)
                        sA = ps.tile([128, 1024], F32, tag="q4", bufs=4)
                        sB = ps.tile([128, 1024], F32, tag="q4", bufs=4)
                        for n in range(2):
                            nsl = slice(n * 512, (n + 1) * 512)
                            qsl = slice(j * JW + n * 512, j * JW + (n + 1) * 512)
                            nc.tensor.matmul(
                                sA[:, nsl], kt[p][:, isl], qtz[hA][:, qsl],
                                start=True, stop=True,
                            )
                            nc.tensor.matmul(
                                sB[:, nsl], kt[p][:, isl], qtz[hB][:, qsl],
                                start=True, stop=True,
                            )
                        eA = exp_pool.tile([128, 1024], BF16, tag="exp")
                        eB = exp_pool.tile([128, 1024], BF16, tag="exp")
                        nc.scalar.activation(eA, sA, mybir.ActivationFunctionType.Exp)
                        nc.scalar.activation(eB, sB, mybir.ActivationFunctionType.Exp)
                        st = dict(start=(i == 0), stop=(i == NI - 1))
                        for n in range(2):
                            nsl = slice(n * 512, (n + 1) * 512)
                            nc.tensor.matmul(
                                avA[:, nsl], v_sb[:, i, hA, :], eA[:, nsl], **st
                            )
                        for n in range(2):
                            nsl = slice(n * 512, (n + 1) * 512)
                            nc.tensor.matmul(
                                avB[:, nsl], v_sb[:, i, hB, :], eB[:, nsl], **st
                            )
                    # drain: denominator (psum parts 0:64) -> approx recip;
                    # values enter the mul as a shifted PSUM operand.
                    recA = rec_pool.tile([64, 1024], F32, tag="rec")
                    recB = rec_pool.tile([64, 1024], F32, tag="rec")
                    nc.vector.reciprocal_approx_fast(out=recA, in_=avA[0:64, :])
                    nc.vector.tensor_mul(send[:, hA, :], avA[64:128, :], recA)
                    nc.vector.reciprocal_approx_fast(out=recB, in_=avB[0:64, :])
                    nc.vector.tensor_mul(send[:, hB, :], avB[64:128, :], recB)

                    # ship this pair: slot r gets our 2 heads for perm-block
                    # r%4. One DMA per (4-slot half, head) - 4 issues instead
                    # of 8 - keeps the ~0.6us-per-issue SP serialization off
                    # the drain -> collective critical path.
                    for g in range(2):
                        for hh in range(2):
                            nc.sync.dma_start(
                                out=a2a_in[j][p][
                                    g * 4 : (g + 1) * 4, hh * DK : (hh + 1) * DK, :
                                ].rearrange("r q c -> q r c"),
                                in_=send[:, hA + hh, :].rearrange(
                                    "q (r c) -> q r c", c=256
                                ),
                            )
                    nc.gpsimd.collective_compute(
                        "AllToAll",
                        mybir.AluOpType.bypass,
                        replica_groups=[list(range(N_CORES))],
                        ins=[a2a_in[j][p][:]],
                        outs=[a2a_out[j][p][:]],
                    )
                # a2l loads ride the Pool queue, emitted per chunk so they
                # fire as soon as this chunk's collectives land (only the next
                # collective trigger queues behind them, with ample slack)
                for p in range(NP):
                    a2lo = a2l_pool.tile(
                        [128, 4, 256], BF16, tag="a2l", bufs=8, name=f"a2lo{j}_{p}"
                    )
                    a2hi = a2l_pool.tile(
                        [128, 4, 256], BF16, tag="a2l", bufs=8, name=f"a2hi{j}_{p}"
                    )
                    a2t[(j, p)] = (a2lo, a2hi)
                    # chunk 1's loads are tail-critical: the SP hwdge queue is
                    # idle then and skips the ~5us Q7 descriptor-generation the
                    # Pool SWDGE path pays after the last collective
                    eng = nc.gpsimd if j == 0 else nc.sync
                    for i in range(4):
                        eng.dma_start(out=a2lo[:, i, :], in_=a2a_out[j][p][i])
                        eng.dma_start(out=a2hi[:, i, :], in_=a2a_out[j][p][4 + i])
                if j == 0:
                    # wo bias broadcast rides the ACT queue behind j0's exps:
                    # transfers during attention when DMA is otherwise idle
                    nc.scalar.dma_start(
                        out=bo_rep,
                        in_=bass.AP(tensor=bo[:].tensor, offset=0, ap=[[0, 128], [1, D]]),
                    )

            # --- output projections: W_O(j0) overlaps attention j1 / A2A tail;
            # a2l loads ride the Pool queue so they fire as soon as each
            # collective lands; accumulation runs pair-major so pair-0 chunks
            # start before the pair-1 A2A completes ---
            for j in range(NJ):
                for p in range(NP):
                    if (j, p) not in mrg:
                        emit_merge(j, p)
                for m in range(2):
                    po = ps.tile([128, 1024], F32, tag="q4", bufs=4)
                    for p in range(NP):
                        for i in range(4):
                            gch = 2 * i + p
                            for n in range(2):
                                nsl = slice(n * 512, (n + 1) * 512)
                                nc.tensor.matmul(
                                    po[:, nsl],
                                    mrg[(j, p)][:, i, m * 128 : (m + 1) * 128],
                                    wo_sb[:, gch, nsl],
                                    start=(p == 0 and i == 0),
                                    stop=(p == NP - 1 and i == 3),
                                )
                    ob = osb_pool.tile([128, D], F32, tag="ob")
                    nc.vector.tensor_add(ob, po, bo_rep)
                    nc.sync.dma_start(out=out[j, m * 128 : (m + 1) * 128, :], in_=ob)

    nc.compile()
    return nc


_NC_CACHE = {}


def _get_nc():
    if "nc" not in _NC_CACHE:
        _NC_CACHE["nc"] = _build_nc()
    return _NC_CACHE["nc"]


def kernel(Q, K, V, W_Q, b_Q, W_K, b_K, W_V, b_V, W_O, b_O, _trace=False):
    Q, K, V = (np.asarray(x, np.float32) for x in (Q, K, V))
    W_Q, W_K, W_V, W_O = (np.asarray(x, np.float32) for x in (W_Q, W_K, W_V, W_O))
    b_Q, b_K, b_V, b_O = (np.asarray(x, np.float32) for x in (b_Q, b_K, b_V, b_O))
    scale = np.float32(1.0 / np.sqrt(DK))

    in_maps = []
    for c in range(N_CORES):
        b, g = c // 4, c % 4
        es = slice(g * EC, (g + 1) * EC)
        in_maps.append(
            {
                "xqt": np.ascontiguousarray(Q[b].T[:, _PERM]).astype(ml_dtypes.bfloat16),
                "xkt": np.ascontiguousarray(K[b].T).astype(ml_dtypes.bfloat16),
                "xvt": np.ascontiguousarray(V[b].T).astype(ml_dtypes.bfloat16),
                "wq": _wlayout(W_Q[:, es] * scale),
                "wk": _wlayout(W_K[:, es]),
                "wv": _wlayout(W_V[:, es]),
                "wo": _wo_global(W_O),
                "msel": np.tile(
                    np.array([[1.0 - b, float(b)]], np.float32), (128, 1)
                ),
                "bq": np.ascontiguousarray(b_Q[es] * scale),
                "bk": np.ascontiguousarray(b_K[es]),
                "bv": np.ascontiguousarray(b_V[es]),
                "bo": b_O,
            }
        )

    nc = _get_nc()
    res = run_bass_kernel_spmd(nc, in_maps, list(range(N_CORES)), trace=_trace)

    full = np.empty((B, S, D), np.float32)
    for c in range(N_CORES):
        b, r = c // 4, c % 4
        chunks = res.results[c]["out"]  # [NJ, 256, D]
        full[b, r * 512 : r * 512 + 256, :] = chunks[0]
        full[b, r * 512 + 256 : (r + 1) * 512, :] = chunks[1]
    if _trace:
        return full, res
    return full
